# revision 7
# baseline (speedup 1.0000x reference)
"""HardMiningLoss TRN2 kernel v2: n=8192, d=512, 8 NeuronCores, data-parallel rows.

Encoding: PSUM accumulates 64*smneg directly via two fp8 DoubleRow matmuls:
  (-8x_i)^T (8x_j)  +  (16*onehot_i)^T (16*onehot_j)  =  64*(4*same - sim)
All mining reductions become single-pass DVE tensor_scalar+reduce ops on the
f16 copy of 64*smneg (op1 is the REDUCE op: max/min/add):
  rowmax' = max(s')                rowmin' = min(s')
  ncnt    = sum[s' < a']           pcnt    = sum[s' > b']
  nrelu'  = W*a' - sum min(s',a')
with a' = rowmax' - 64*3.9, b' = rowmin' + 64*3.9 computed on the Pool engine.
The pos-side relu sum must be a genuine ACT Relu (small element values): the
max(s',b') algebra accumulates ~8192*250 and loses the answer to f16/f32
rounding (measured +19 bias). The neg side is safe: a' is small for this data.
PSUM evacuation (f32->f16) runs on the Scalar(ACT) engine (Copy). The last-row
mean_pos/mean_neg stats are computed on the host in f64 closed form.
"""
import numpy as np
import ml_dtypes
from contextlib import ExitStack

import concourse.bass as bass
import concourse.tile as tile
from concourse import bacc, mybir
from concourse.bass_utils import run_bass_kernel_spmd

F32 = mybir.dt.float32
F16 = mybir.dt.float16
F8 = mybir.dt.float8e4
Alu = mybir.AluOpType
Act = mybir.ActivationFunctionType
DR = mybir.MatmulPerfMode.DoubleRow

N_TOT, D, N_CORES = 8192, 512, 8
ROWS = N_TOT // N_CORES          # 1024 rows per core
CHUNKS = ROWS // 128             # 8 chunks of 128 rows
QCOLS = 2048                     # quarter-chunk column width (half PSUM x2 bufs)
NQ = N_TOT // QCOLS              # 4 quarters per chunk
KS = D // 128                    # 4 contraction sub-tiles of 128
MARGIN = 0.1
SCL = 64.0                       # (8x)*(8x) scale on sim; 16^2 = 64*4 on same
# kept for test.py compat; the last-row self-pair decision is data-driven now
INCLUDE_SELF_LAST_ROW = True

# stage column layout: 8 cols per quantity (one per chunk)
C_MAX, C_MIN, C_NCNT, C_PCNT, C_PRELU, C_MINS = 0, 8, 16, 24, 32, 40
STAGE_W = 48


def build_program():
    nc = bacc.Bacc("TRN2", target_bir_lowering=False, debug=False)
    x8_d = nc.dram_tensor("x8", [128, KS, N_TOT], F8, kind="ExternalInput")
    H8_d = nc.dram_tensor("H8", [128, KS, N_TOT], F8, kind="ExternalInput")
    xn8_d = nc.dram_tensor("xn8", [128, KS, ROWS], F8, kind="ExternalInput")
    h8_d = nc.dram_tensor("h8", [128, KS, ROWS], F8, kind="ExternalInput")
    st_d = nc.dram_tensor("stage", [128, STAGE_W], F32, kind="ExternalOutput")

    with tile.TileContext(nc) as tc, ExitStack() as ctx:
        pool = ctx.enter_context(tc.tile_pool(name="p", bufs=1))
        dbuf = ctx.enter_context(tc.tile_pool(name="db", bufs=2))
        pspool = ctx.enter_context(
            tc.tile_pool(name="ps", bufs=2, space=bass.MemorySpace.PSUM))

        x8 = pool.tile([128, KS, N_TOT], F8)
        H8 = pool.tile([128, KS, N_TOT], F8)
        xn8 = pool.tile([128, KS, ROWS], F8)
        h8 = pool.tile([128, KS, ROWS], F8)
        junkD = pool.tile([128, N_TOT], F16)   # DVE elementwise dump
        junkA = pool.tile([128, N_TOT], F16)   # ACT relu dump
        stage = pool.tile([128, STAGE_W], F32)
        alpha = pool.tile([128, CHUNKS], F32)
        beta = pool.tile([128, CHUNKS], F32)
        bneg = pool.tile([128, CHUNKS], F32)

        nc.sync.dma_start(xn8[:], xn8_d.ap())
        nc.sync.dma_start(h8[:], h8_d.ap())
        for q in range(NQ):
            cs = slice(q * QCOLS, (q + 1) * QCOLS)
            nc.sync.dma_start(x8[:, :, cs], x8_d.ap()[:, :, cs])
            nc.sync.dma_start(H8[:, :, cs], H8_d.ap()[:, :, cs])

        for c in range(CHUNKS):
            smneg = dbuf.tile([128, N_TOT], F16, name="smneg")
            rsl = slice(c * 128, (c + 1) * 128)
            for q in range(NQ):
                ps = pspool.tile([128, QCOLS], F32)
                for kk in range(KS // 2):
                    ks = slice(2 * kk, 2 * kk + 2)
                    for nb in range(QCOLS // 512):
                        col = q * QCOLS + nb * 512
                        nc.tensor.matmul(
                            ps[:, nb * 512:(nb + 1) * 512],
                            xn8[:, ks, rsl], x8[:, ks, col:col + 512],
                            start=(kk == 0), stop=False, perf_mode=DR)
                for kk in range(KS // 2):
                    ks = slice(2 * kk, 2 * kk + 2)
                    for nb in range(QCOLS // 512):
                        col = q * QCOLS + nb * 512
                        nc.tensor.matmul(
                            ps[:, nb * 512:(nb + 1) * 512],
                            h8[:, ks, rsl], H8[:, ks, col:col + 512],
                            start=False, stop=(kk == KS // 2 - 1), perf_mode=DR)
                # ACT evacuation PSUM f32 -> SBUF f16 (keeps 64x scale)
                nc.scalar.activation(smneg[:, q * QCOLS:(q + 1) * QCOLS], ps[:],
                                     Act.Copy, bias=0.0, scale=1.0)

            # DVE single-pass reductions (op1 = reduce op, scalar2 = init)
            nc.vector.tensor_scalar(junkD[:], smneg[:], 0.0, -1e30,
                                    Alu.add, Alu.max,
                                    accum_out=stage[:, C_MAX + c:C_MAX + c + 1])
            nc.vector.tensor_scalar(junkD[:], smneg[:], 0.0, 1e30,
                                    Alu.add, Alu.min,
                                    accum_out=stage[:, C_MIN + c:C_MIN + c + 1])
            # thresholds on Pool: a' = max' - 64*3.9 ; b' = min' + 64*3.9
            nc.gpsimd.tensor_scalar(alpha[:, c:c + 1],
                                    stage[:, C_MAX + c:C_MAX + c + 1],
                                    -SCL * (4.0 - MARGIN), None, Alu.add)
            nc.gpsimd.tensor_scalar(beta[:, c:c + 1],
                                    stage[:, C_MIN + c:C_MIN + c + 1],
                                    SCL * (4.0 - MARGIN), None, Alu.add)
            nc.gpsimd.tensor_scalar(bneg[:, c:c + 1],
                                    stage[:, C_MIN + c:C_MIN + c + 1],
                                    -1.0, -SCL * (4.0 - MARGIN),
                                    Alu.mult, Alu.add)
            a_ap = alpha[:, c:c + 1]
            b_ap = beta[:, c:c + 1]
            nc.vector.tensor_scalar(junkD[:], smneg[:], a_ap, 0.0,
                                    Alu.is_lt, Alu.add,
                                    accum_out=stage[:, C_NCNT + c:C_NCNT + c + 1])
            nc.vector.tensor_scalar(junkD[:], smneg[:], b_ap, 0.0,
                                    Alu.is_gt, Alu.add,
                                    accum_out=stage[:, C_PCNT + c:C_PCNT + c + 1])
            # prelu' = sum relu(s' - b') on ACT (elements small and sparse)
            nc.scalar.activation(junkA[:], smneg[:], Act.Relu,
                                 bias=bneg[:, c:c + 1], scale=1.0,
                                 accum_out=stage[:, C_PRELU + c:C_PRELU + c + 1])
            # nrelu' = W*a' - sum min(s',a')   (host does the W*a' part)
            nc.vector.tensor_scalar(junkD[:], smneg[:], a_ap,
                                    0.0, Alu.min, Alu.add,
                                    accum_out=stage[:, C_MINS + c:C_MINS + c + 1])

        nc.sync.dma_start(st_d.ap(), stage[:])
    nc.compile()
    return nc


_NC_CACHE = None


def kernel(inputs, targets, _want_time=False, _trace=False):
    global _NC_CACHE
    x = np.asarray(inputs, dtype=np.float32)          # [N, D]
    tgt = np.asarray(targets).astype(np.int64)        # [N]

    # fp8 operands (shared, unrotated: moving j-axis order is global)
    xT = np.ascontiguousarray(x.T)                    # [D, N]
    x8 = np.ascontiguousarray(
        (8.0 * xT).reshape(KS, 128, N_TOT).transpose(1, 0, 2)
    ).astype(ml_dtypes.float8_e4m3)                   # [128, KS, N]
    H = np.zeros((D, N_TOT), dtype=np.float32)
    H[tgt, np.arange(N_TOT)] = 16.0
    H8 = np.ascontiguousarray(
        H.reshape(KS, 128, N_TOT).transpose(1, 0, 2)
    ).astype(ml_dtypes.float8_e4m3)

    if _NC_CACHE is None:
        _NC_CACHE = build_program()
    nc = _NC_CACHE

    in_maps = []
    for m in range(N_CORES):
        own = slice(m * ROWS, (m + 1) * ROWS)
        xn8_m = np.ascontiguousarray(
            (-x8[:, :, own].astype(np.float32))).astype(ml_dtypes.float8_e4m3)
        h8_m = np.ascontiguousarray(H8[:, :, own])
        in_maps.append({"x8": x8, "H8": H8, "xn8": xn8_m, "h8": h8_m})

    res = run_bass_kernel_spmd(nc, in_maps, core_ids=list(range(N_CORES)),
                               trace=_trace)

    # ---- host finisher ----
    n = N_TOT
    W = float(N_TOT)
    maxS = np.empty(n); minS = np.empty(n)
    ncnt = np.empty(n); pcnt = np.empty(n)
    prelu = np.empty(n); minsum = np.empty(n)
    for m in range(N_CORES):
        st = np.asarray(res.results[m]["stage"], dtype=np.float64)
        for c in range(CHUNKS):
            rows = slice(m * ROWS + c * 128, m * ROWS + (c + 1) * 128)
            maxS[rows] = st[:, C_MAX + c] / SCL
            minS[rows] = st[:, C_MIN + c] / SCL
            ncnt[rows] = st[:, C_NCNT + c]
            pcnt[rows] = st[:, C_PCNT + c]
            prelu[rows] = st[:, C_PRELU + c] / SCL
            minsum[rows] = st[:, C_MINS + c] / SCL

    ncnt = np.round(ncnt)
    pcnt = np.round(pcnt)
    alpha = maxS - (4.0 - MARGIN)
    beta = minS + (4.0 - MARGIN)
    nrelu = W * alpha - minsum
    # neg: kept s < alpha ; nrelu = alpha*ncnt - sum(s_kept); sim = -s
    neg_sum_sim = nrelu - alpha * ncnt
    # pos: kept s > beta ; prelu = sum(s_kept) - beta*pcnt; sim = 4 - s
    pos_sum_s = prelu + beta * pcnt
    pos_sum_sim = 4.0 * pcnt - pos_sum_s

    pos_loss = (pcnt - pos_sum_sim) / np.maximum(pcnt, 1.0)
    neg_loss = neg_sum_sim / np.maximum(ncnt, 1.0)
    valid = ncnt >= 1.0
    loss = np.sum(np.where(valid, pos_loss + neg_loss, 0.0)) / n
    prec = np.sum(~valid) / n

    # last-row unmined stats: exact f64 closed form on host
    c_last = tgt[-1]
    xl = x[-1].astype(np.float64)
    x64 = x.astype(np.float64)
    same_l = tgt == c_last
    sum_all = x64.sum(axis=0) @ xl
    sum_same = x64[same_l].sum(axis=0) @ xl          # includes self
    self_sim = float(xl @ xl)
    include_self = np.float32(self_sim) < np.float32(1.0)
    pos_cnt_all = int(same_l.sum()) - 1 + (1 if include_self else 0)
    pos_sim_all = sum_same - (0.0 if include_self else self_sim)
    neg_cnt_all = int((~same_l).sum())
    neg_sim_all = sum_all - sum_same
    mean_pos_sim = pos_sim_all / max(pos_cnt_all, 1)
    mean_neg_sim = neg_sim_all / max(neg_cnt_all, 1)

    out = np.array([loss, prec, mean_pos_sim, mean_neg_sim], dtype=np.float32)
    if _want_time:
        return out, res
    return out


# revision 33
# speedup vs baseline: 1.9912x; 1.9912x over previous
"""HardMiningLoss TRN2 kernel v5: n=8192, d=512, 8 NeuronCores, data-parallel rows.

Encoding: PSUM accumulates 64*smneg = 64*(4*same - sim) via fp8 DoubleRow
matmuls: (-8x_i)^T(8x_j) + (16*onehot_i)^T(16*onehot_j).

Class-sorted layout: columns (and row blocks) are ordered by class, rotated
per core so its own rows sit at local columns [0,1024). All of a chunk's
same-class columns then live in the fixed window [c*128-64, c*128+192) —
guaranteed as long as every class has <= 64 members (asserted; ~16 expected).
Since positives (s' >= ~190) and negatives (|s'| <= ~70) are separated by the
pos threshold b' >= ~185, the onehot matmul, rowmax, pcnt and the pos relu sum
only need the 1-2 512-col blocks covering the window. Only rowmin, ncnt and
the neg min-sum touch all 8192 columns.

Per-row reductions are single-pass DVE tensor_scalar ops (op1 = REDUCE op):
  rowmin' = min(s')   ncnt = sum[s' < a']   nrelu' = W*a' - sum min(s',a')
window:  rowmax' = max  pcnt = sum[s' > b']  prelu' = sum relu(s'-b') (ACT)
a' = rowmax' - 64*3.9, b' = rowmin' + 64*3.9 on the Pool engine. The pos relu
must be a genuine ACT Relu: sum-of-max algebra at threshold ~250 loses the
answer to rounding. PSUM evacuation (f32->f16) is ACT Copy; the ACT relu for
chunk c-1 is emitted after chunk c's evacs to avoid head-of-line blocking.
Last-row mean_pos/mean_neg stats are computed on the host in f64 closed form.
"""
import numpy as np
import ml_dtypes
from contextlib import ExitStack

import concourse.bass as bass
import concourse.tile as tile
from concourse import bacc, mybir
from concourse.bass_utils import run_bass_kernel_spmd

F32 = mybir.dt.float32
F16 = mybir.dt.float16
F8 = mybir.dt.float8e4
Alu = mybir.AluOpType
Act = mybir.ActivationFunctionType
DR = mybir.MatmulPerfMode.DoubleRow

N_TOT, D, N_CORES = 8192, 512, 8
ROWS = N_TOT // N_CORES          # 1024 rows per core
CHUNKS = ROWS // 128             # 8 chunks of 128 rows
QCOLS = 2048                     # quarter-chunk column width (half PSUM x2 bufs)
NQ = N_TOT // QCOLS              # 4 quarters per chunk
KS = D // 128                    # 4 contraction sub-tiles of 128
NBLK = N_TOT // 512              # 16 column blocks of 512
MARGIN = 0.1
SCL = 64.0                       # (8x)*(8x) scale on sim; 16^2 = 64*4 on same
MAX_CLASS = 64                   # window pad; host asserts class sizes <= this
# kept for test.py compat; the last-row self-pair decision is data-driven now
INCLUDE_SELF_LAST_ROW = True

# stage column layout (full quantities: 8 cols; window quantities: 2x8 cols)
C_MIN, C_NCNT, C_MINS = 0, 8, 16
C_MAX, C_PCNT, C_PRELU = 24, 40, 56
C_SGN, C_NRELU = 72, 73         # last-chunk tail split (ACT halves)
STAGE_W = 74
TSPL = 6656                     # last-chunk col split: DVE [0:TSPL], ACT rest


def win_blocks(c):
    """512-col blocks (local coords) covering [c*128-64, c*128+192)."""
    w0 = (c * 128 - MAX_CLASS) % N_TOT
    w1 = (c * 128 + 128 + MAX_CLASS - 1) % N_TOT
    b0, b1 = w0 // 512, w1 // 512
    return [b0] if b0 == b1 else [b0, b1]


def win_pieces(c):
    """Exact col ranges (local coords) of the window [c*128-64, c*128+192)."""
    lo = c * 128 - MAX_CLASS
    hi = c * 128 + 128 + MAX_CLASS
    if lo < 0:
        return [(0, hi), (N_TOT + lo, N_TOT)]
    return [(lo, hi)]


# blocks of H8 (onehot moving operand) ever needed, in local coords
H_BLOCKS = sorted({b for c in range(CHUNKS) for b in win_blocks(c)},
                  key=lambda b: (b + 1) % NBLK)          # [15, 0, 1, 2]
H_MAP = {b: i for i, b in enumerate(H_BLOCKS)}
NHB = len(H_BLOCKS)


def build_program():
    nc = bacc.Bacc("TRN2", target_bir_lowering=False, debug=False)
    x8_d = nc.dram_tensor("x8", [128, KS, N_TOT], F8, kind="ExternalInput")
    H8_d = nc.dram_tensor("H8", [128, KS, NHB * 512], F8, kind="ExternalInput")
    xn8_d = nc.dram_tensor("xn8", [128, KS, ROWS], F8, kind="ExternalInput")
    st_d = nc.dram_tensor("stage", [128, STAGE_W], F32, kind="ExternalOutput")

    with tile.TileContext(nc) as tc, ExitStack() as ctx:
        pool = ctx.enter_context(tc.tile_pool(name="p", bufs=1))
        dbuf = ctx.enter_context(tc.tile_pool(name="db", bufs=2))
        pspool = ctx.enter_context(
            tc.tile_pool(name="ps", bufs=2, space=bass.MemorySpace.PSUM))

        x8 = pool.tile([128, KS, N_TOT], F8)
        H8 = pool.tile([128, KS, NHB * 512], F8)
        xn8 = pool.tile([128, KS, ROWS], F8)
        junkD = pool.tile([128, N_TOT], F16)   # DVE elementwise dump
        junkA = pool.tile([128, N_TOT], F16)   # ACT elementwise dump
        stage = pool.tile([128, STAGE_W], F32)
        alpha = pool.tile([128, CHUNKS], F32)
        beta = pool.tile([128, CHUNKS], F32)
        bneg = pool.tile([128, CHUNKS], F32)
        aneg = pool.tile([128, 1], F32)
        # own-row onehot stationary = H8 window blocks at local cols [0,1024)
        assert H_MAP[0] == 1 and H_MAP[1] == 2

        nc.vector.memset(stage[:], 0.0)
        # DMA order: get chunk-0 matmul inputs on chip first
        nc.sync.dma_start(xn8[:], xn8_d.ap())
        nc.sync.dma_start(x8[:, :, 0:QCOLS], x8_d.ap()[:, :, 0:QCOLS])
        nc.sync.dma_start(H8[:, :, 0:1024], H8_d.ap()[:, :, 0:1024])
        for q in range(1, NQ):
            cs = slice(q * QCOLS, (q + 1) * QCOLS)
            nc.sync.dma_start(x8[:, :, cs], x8_d.ap()[:, :, cs])
        nc.sync.dma_start(H8[:, :, 1024:NHB * 512],
                          H8_d.ap()[:, :, 1024:NHB * 512])

        smnegs = []
        for c in range(CHUNKS):
            smneg = dbuf.tile([128, N_TOT], F16, name="smneg")
            smnegs.append(smneg)
            rsl = slice(c * 128, (c + 1) * 128)
            wset = set(win_blocks(c))
            for q in range(NQ):
                ps = pspool.tile([128, QCOLS], F32)
                for kk in range(KS // 2):
                    ks = slice(2 * kk, 2 * kk + 2)
                    for nb in range(QCOLS // 512):
                        B = q * 4 + nb
                        col = B * 512
                        nc.tensor.matmul(
                            ps[:, nb * 512:(nb + 1) * 512],
                            xn8[:, ks, rsl], x8[:, ks, col:col + 512],
                            start=(kk == 0),
                            stop=(kk == KS // 2 - 1 and B not in wset),
                            perf_mode=DR)
                # onehot matmuls only for the window blocks of this quarter
                for kk in range(KS // 2):
                    ks = slice(2 * kk, 2 * kk + 2)
                    for nb in range(QCOLS // 512):
                        B = q * 4 + nb
                        if B not in wset:
                            continue
                        hcol = H_MAP[B] * 512
                        nc.tensor.matmul(
                            ps[:, nb * 512:(nb + 1) * 512],
                            H8[:, ks, 512 + c * 128:512 + (c + 1) * 128],
                            H8[:, ks, hcol:hcol + 512],
                            start=False, stop=(kk == KS // 2 - 1),
                            perf_mode=DR)
                # evacuation PSUM f32 -> SBUF f16 (keeps 64x scale):
                # ACT does all but the last 512 cols of q3, DVE does those
                if q < NQ - 1:
                    nc.scalar.activation(smneg[:, q * QCOLS:(q + 1) * QCOLS],
                                         ps[:], Act.Copy, bias=0.0, scale=1.0)
                else:
                    nc.scalar.activation(
                        smneg[:, q * QCOLS:(q + 1) * QCOLS - 512],
                        ps[:, 0:QCOLS - 512], Act.Copy, bias=0.0, scale=1.0)
                    nc.vector.tensor_scalar(
                        smneg[:, (q + 1) * QCOLS - 512:(q + 1) * QCOLS],
                        ps[:, QCOLS - 512:QCOLS], 1.0, None, Alu.mult)

            # rowmin first: it feeds bneg -> relu-p, the longest chain
            nc.vector.tensor_scalar(junkD[:], smneg[:], 0.0, 1e30,
                                    Alu.add, Alu.min,
                                    accum_out=stage[:, C_MIN + c:C_MIN + c + 1])
            # rowmax over the exact class window only (positives >> negatives)
            for i, (lo, hi) in enumerate(win_pieces(c)):
                nc.vector.tensor_scalar(
                    junkD[:, lo:hi], smneg[:, lo:hi], 0.0, -1e30,
                    Alu.add, Alu.max,
                    accum_out=stage[:, C_MAX + 2 * c + i:C_MAX + 2 * c + i + 1])
            # bneg on Pool (feeds the ACT relu next chunk); a'/b' on DVE so
            # the count passes don't wait on a cross-engine hop
            nc.gpsimd.tensor_scalar(bneg[:, c:c + 1],
                                    stage[:, C_MIN + c:C_MIN + c + 1],
                                    -1.0, -SCL * (4.0 - MARGIN),
                                    Alu.mult, Alu.add)
            nc.vector.tensor_scalar(beta[:, c:c + 1],
                                    stage[:, C_MIN + c:C_MIN + c + 1],
                                    SCL * (4.0 - MARGIN), None, Alu.add)
            if len(win_pieces(c)) == 1:
                nc.vector.tensor_scalar(alpha[:, c:c + 1],
                                        stage[:, C_MAX + 2 * c:C_MAX + 2 * c + 1],
                                        -SCL * (4.0 - MARGIN), None, Alu.add)
            else:
                nc.vector.tensor_scalar(alpha[:, c:c + 1],
                                        stage[:, C_MAX + 2 * c:C_MAX + 2 * c + 1],
                                        stage[:, C_MAX + 2 * c + 1:C_MAX + 2 * c + 2],
                                        None, Alu.max)
                nc.vector.tensor_scalar(alpha[:, c:c + 1], alpha[:, c:c + 1],
                                        -SCL * (4.0 - MARGIN), None, Alu.add)
            # software pipeline: ACT relu-p (window) for chunk c-1 after
            # this chunk's evacs so it doesn't head-of-line block them
            if c >= 1:
                cp = c - 1
                for i, (lo, hi) in enumerate(win_pieces(cp)):
                    nc.scalar.activation(
                        junkA[:, lo:hi], smnegs[cp][:, lo:hi], Act.Relu,
                        bias=bneg[:, cp:cp + 1], scale=1.0,
                        accum_out=stage[:, C_PRELU + 2 * cp + i:
                                        C_PRELU + 2 * cp + i + 1])
            a_ap = alpha[:, c:c + 1]
            b_ap = beta[:, c:c + 1]
            last = c == CHUNKS - 1
            ncols = TSPL if last else N_TOT
            if last:
                nc.vector.tensor_scalar(aneg[:], a_ap, -1.0, None, Alu.mult)
            nc.vector.tensor_scalar(junkD[:, 0:ncols], smneg[:, 0:ncols],
                                    a_ap, 0.0, Alu.is_lt, Alu.add,
                                    accum_out=stage[:, C_NCNT + c:C_NCNT + c + 1])
            for i, (lo, hi) in enumerate(win_pieces(c)):
                nc.vector.tensor_scalar(
                    junkD[:, lo:hi], smneg[:, lo:hi], b_ap, 0.0,
                    Alu.is_gt, Alu.add,
                    accum_out=stage[:, C_PCNT + 2 * c + i:C_PCNT + 2 * c + i + 1])
            # nrelu' = W*a' - sum min(s',a')   (host does the W*a' part)
            nc.vector.tensor_scalar(junkD[:, 0:ncols], smneg[:, 0:ncols], a_ap,
                                    0.0, Alu.min, Alu.add,
                                    accum_out=stage[:, C_MINS + c:C_MINS + c + 1])
            if last:
                # tail split: ACT covers cols [TSPL:W] of the last chunk so
                # the final chunk's reductions run on both engines.
                # sum sign(s'-a') -> count part; sum relu(a'-s') -> nrelu part
                nc.scalar.activation(
                    junkA[:, TSPL:], smneg[:, TSPL:], Act.Sign,
                    bias=aneg[:], scale=1.0,
                    accum_out=stage[:, C_SGN:C_SGN + 1])
                nc.scalar.activation(
                    junkA[:, TSPL:], smneg[:, TSPL:], Act.Relu,
                    bias=a_ap, scale=-1.0,
                    accum_out=stage[:, C_NRELU:C_NRELU + 1])

        c = CHUNKS - 1
        for i, (lo, hi) in enumerate(win_pieces(c)):
            nc.scalar.activation(
                junkA[:, lo:hi], smnegs[c][:, lo:hi], Act.Relu,
                bias=bneg[:, c:c + 1], scale=1.0,
                accum_out=stage[:, C_PRELU + 2 * c + i:C_PRELU + 2 * c + i + 1])

        nc.sync.dma_start(st_d.ap(), stage[:])
    nc.compile()
    return nc


_NC_CACHE = None


def kernel(inputs, targets, _want_time=False, _trace=False):
    global _NC_CACHE
    x = np.asarray(inputs, dtype=np.float32)          # [N, D]
    tgt = np.asarray(targets).astype(np.int64)        # [N]

    # class-sorted permutation of rows/columns
    perm = np.argsort(tgt, kind="stable")
    xs = x[perm]
    ts_ = tgt[perm]
    assert np.bincount(tgt).max() <= MAX_CLASS, "class too large for window"

    xT = np.ascontiguousarray(xs.T)                   # [D, N] sorted cols
    x8g = np.ascontiguousarray(
        (8.0 * xT).reshape(KS, 128, N_TOT).transpose(1, 0, 2)
    ).astype(ml_dtypes.float8_e4m3)                   # [128, KS, N]
    H = np.zeros((D, N_TOT), dtype=np.float32)
    H[ts_, np.arange(N_TOT)] = 16.0
    H8g = np.ascontiguousarray(
        H.reshape(KS, 128, N_TOT).transpose(1, 0, 2)
    ).astype(ml_dtypes.float8_e4m3)

    if _NC_CACHE is None:
        _NC_CACHE = build_program()
    nc = _NC_CACHE

    in_maps = []
    for m in range(N_CORES):
        sh = m * ROWS
        x8m = np.roll(x8g, -sh, axis=2)
        xn8m = np.ascontiguousarray(
            (-x8m[:, :, 0:ROWS].astype(np.float32))).astype(
                ml_dtypes.float8_e4m3)
        # H8 window blocks in local coords -> global cols (sh + b*512) % N
        hw = np.concatenate(
            [np.take(H8g, (np.arange(b * 512, (b + 1) * 512) + sh) % N_TOT,
                     axis=2) for b in H_BLOCKS], axis=2)
        in_maps.append({"x8": np.ascontiguousarray(x8m),
                        "H8": np.ascontiguousarray(hw), "xn8": xn8m})

    res = run_bass_kernel_spmd(nc, in_maps, core_ids=list(range(N_CORES)),
                               trace=_trace)

    # ---- host finisher (rows are in sorted order; loss is order-invariant) --
    n = N_TOT
    W = float(N_TOT)
    minS = np.empty(n); maxS = np.empty(n)
    ncnt = np.empty(n); pcnt = np.empty(n)
    prelu = np.empty(n); minsum = np.empty(n)
    for m in range(N_CORES):
        st = np.asarray(res.results[m]["stage"], dtype=np.float64)
        for c in range(CHUNKS):
            rows = slice(m * ROWS + c * 128, m * ROWS + (c + 1) * 128)
            nb = len(win_pieces(c))
            minS[rows] = st[:, C_MIN + c] / SCL
            ncnt[rows] = st[:, C_NCNT + c]
            minsum[rows] = st[:, C_MINS + c] / SCL
            mx = st[:, C_MAX + 2 * c]
            if nb == 2:
                mx = np.maximum(mx, st[:, C_MAX + 2 * c + 1])
            maxS[rows] = mx / SCL
            pc = st[:, C_PCNT + 2 * c]
            pr = st[:, C_PRELU + 2 * c]
            if nb == 2:
                pc = pc + st[:, C_PCNT + 2 * c + 1]
                pr = pr + st[:, C_PRELU + 2 * c + 1]
            pcnt[rows] = pc
            prelu[rows] = pr / SCL

    ncnt = np.round(ncnt)
    pcnt = np.round(pcnt)
    alpha = maxS - (4.0 - MARGIN)
    beta = minS + (4.0 - MARGIN)
    nrelu = W * alpha - minsum
    # last chunk of each core: DVE covered cols [0:TSPL], ACT the rest
    for m in range(N_CORES):
        st = np.asarray(res.results[m]["stage"], dtype=np.float64)
        rows = slice(m * ROWS + (CHUNKS - 1) * 128, (m + 1) * ROWS)
        a_r = alpha[rows]
        ncnt[rows] = np.round(
            ncnt[rows] + ((N_TOT - TSPL) - st[:, C_SGN]) / 2.0)
        nrelu[rows] = (TSPL * a_r - minsum[rows]) + st[:, C_NRELU] / SCL
    # neg: kept s < alpha ; nrelu = alpha*ncnt - sum(s_kept); sim = -s
    neg_sum_sim = nrelu - alpha * ncnt
    # pos: kept s > beta ; prelu = sum(s_kept) - beta*pcnt; sim = 4 - s
    pos_sum_sim = 4.0 * pcnt - (prelu + beta * pcnt)

    pos_loss = (pcnt - pos_sum_sim) / np.maximum(pcnt, 1.0)
    neg_loss = neg_sum_sim / np.maximum(ncnt, 1.0)
    valid = ncnt >= 1.0
    loss = np.sum(np.where(valid, pos_loss + neg_loss, 0.0)) / n
    prec = np.sum(~valid) / n

    # last-row unmined stats: exact f64 closed form on host (original order)
    c_last = tgt[-1]
    xl = x[-1].astype(np.float64)
    x64 = x.astype(np.float64)
    same_l = tgt == c_last
    sum_all = x64.sum(axis=0) @ xl
    sum_same = x64[same_l].sum(axis=0) @ xl          # includes self
    self_sim = float(xl @ xl)
    include_self = np.float32(self_sim) < np.float32(1.0)
    pos_cnt_all = int(same_l.sum()) - 1 + (1 if include_self else 0)
    pos_sim_all = sum_same - (0.0 if include_self else self_sim)
    neg_cnt_all = int((~same_l).sum())
    neg_sim_all = sum_all - sum_same
    mean_pos_sim = pos_sim_all / max(pos_cnt_all, 1)
    mean_neg_sim = neg_sim_all / max(neg_cnt_all, 1)

    out = np.array([loss, prec, mean_pos_sim, mean_neg_sim], dtype=np.float32)
    if _want_time:
        return out, res
    return out


# revision 41
# speedup vs baseline: 2.0107x; 1.0098x over previous
"""HardMiningLoss TRN2 kernel v5: n=8192, d=512, 8 NeuronCores, data-parallel rows.

Encoding: PSUM accumulates 64*smneg = 64*(4*same - sim) via fp8 DoubleRow
matmuls: (-8x_i)^T(8x_j) + (16*onehot_i)^T(16*onehot_j).

Class-sorted layout: columns (and row blocks) are ordered by class, rotated
per core so its own rows sit at local columns [0,1024). All of a chunk's
same-class columns then live in the fixed window [c*128-64, c*128+192) —
guaranteed as long as every class has <= 64 members (asserted; ~16 expected).
Since positives (s' >= ~190) and negatives (|s'| <= ~70) are separated by the
pos threshold b' >= ~185, the onehot matmul, rowmax, pcnt and the pos relu sum
only need the 1-2 512-col blocks covering the window. Only rowmin, ncnt and
the neg min-sum touch all 8192 columns.

Per-row reductions are single-pass DVE tensor_scalar ops (op1 = REDUCE op):
  rowmin' = min(s')   ncnt = sum[s' < a']   nrelu' = W*a' - sum min(s',a')
window:  rowmax' = max  pcnt = sum[s' > b']  prelu' = sum relu(s'-b') (ACT)
a' = rowmax' - 64*3.9, b' = rowmin' + 64*3.9 on the Pool engine. The pos relu
must be a genuine ACT Relu: sum-of-max algebra at threshold ~250 loses the
answer to rounding. PSUM evacuation (f32->f16) is ACT Copy; the ACT relu for
chunk c-1 is emitted after chunk c's evacs to avoid head-of-line blocking.
Last-row mean_pos/mean_neg stats are computed on the host in f64 closed form.
"""
import numpy as np
import ml_dtypes
from contextlib import ExitStack

import concourse.bass as bass
import concourse.tile as tile
from concourse import bacc, mybir
from concourse.bass_utils import run_bass_kernel_spmd

F32 = mybir.dt.float32
F16 = mybir.dt.float16
F8 = mybir.dt.float8e4
Alu = mybir.AluOpType
Act = mybir.ActivationFunctionType
DR = mybir.MatmulPerfMode.DoubleRow

N_TOT, D, N_CORES = 8192, 512, 8
ROWS = N_TOT // N_CORES          # 1024 rows per core
CHUNKS = ROWS // 128             # 8 chunks of 128 rows
QCOLS = 2048                     # quarter-chunk column width (half PSUM x2 bufs)
NQ = N_TOT // QCOLS              # 4 quarters per chunk
KS = D // 128                    # 4 contraction sub-tiles of 128
NBLK = N_TOT // 512              # 16 column blocks of 512
MARGIN = 0.1
SCL = 64.0                       # (8x)*(8x) scale on sim; 16^2 = 64*4 on same
MAX_CLASS = 64                   # window pad; host asserts class sizes <= this
# kept for test.py compat; the last-row self-pair decision is data-driven now
INCLUDE_SELF_LAST_ROW = True

# stage column layout (full quantities: 8 cols; window quantities: 2x8 cols)
C_MIN, C_NCNT, C_MINS = 0, 8, 16
C_MAX, C_PCNT, C_PRELU = 24, 40, 56
C_SGN, C_NRELU = 72, 73         # last-chunk tail split (ACT halves)
STAGE_W = 74
TSPL = 6656                     # last-chunk col split: DVE [0:TSPL], ACT rest
DEV = 512                       # evac cols of q3 done by DVE (rest on ACT)


def win_blocks(c):
    """512-col blocks (local coords) covering [c*128-64, c*128+192)."""
    w0 = (c * 128 - MAX_CLASS) % N_TOT
    w1 = (c * 128 + 128 + MAX_CLASS - 1) % N_TOT
    b0, b1 = w0 // 512, w1 // 512
    return [b0] if b0 == b1 else [b0, b1]


def win_pieces(c):
    """Exact col ranges (local coords) of the window [c*128-64, c*128+192)."""
    lo = c * 128 - MAX_CLASS
    hi = c * 128 + 128 + MAX_CLASS
    if lo < 0:
        return [(0, hi), (N_TOT + lo, N_TOT)]
    return [(lo, hi)]


# blocks of H8 (onehot moving operand) ever needed, in local coords
H_BLOCKS = sorted({b for c in range(CHUNKS) for b in win_blocks(c)},
                  key=lambda b: (b + 1) % NBLK)          # [15, 0, 1, 2]
H_MAP = {b: i for i, b in enumerate(H_BLOCKS)}
NHB = len(H_BLOCKS)


def build_program():
    nc = bacc.Bacc("TRN2", target_bir_lowering=False, debug=False)
    x8_d = nc.dram_tensor("x8", [128, KS, N_TOT], F8, kind="ExternalInput")
    H8_d = nc.dram_tensor("H8", [128, KS, NHB * 512], F8, kind="ExternalInput")
    xn8_d = nc.dram_tensor("xn8", [128, KS, ROWS], F8, kind="ExternalInput")
    st_d = nc.dram_tensor("stage", [128, STAGE_W], F32, kind="ExternalOutput")

    with tile.TileContext(nc) as tc, ExitStack() as ctx:
        pool = ctx.enter_context(tc.tile_pool(name="p", bufs=1))
        dbuf = ctx.enter_context(tc.tile_pool(name="db", bufs=3))
        pspool = ctx.enter_context(
            tc.tile_pool(name="ps", bufs=2, space=bass.MemorySpace.PSUM))

        x8 = pool.tile([128, KS, N_TOT], F8)
        H8 = pool.tile([128, KS, NHB * 512], F8)
        xn8 = pool.tile([128, KS, ROWS], F8)
        junkD = pool.tile([128, N_TOT], F16)   # DVE elementwise dump
        junkA = pool.tile([128, N_TOT], F16)   # ACT elementwise dump
        stage = pool.tile([128, STAGE_W], F32)
        alpha = pool.tile([128, CHUNKS], F32)
        beta = pool.tile([128, CHUNKS], F32)
        bneg = pool.tile([128, CHUNKS], F32)
        aneg = pool.tile([128, 1], F32)
        qmin = pool.tile([128, NQ], F32)
        # own-row onehot stationary = H8 window blocks at local cols [0,1024)
        assert H_MAP[0] == 1 and H_MAP[1] == 2

        nc.vector.memset(stage[:], 0.0)
        # DMA order: get chunk-0 matmul inputs on chip first
        nc.sync.dma_start(xn8[:], xn8_d.ap())
        nc.sync.dma_start(x8[:, :, 0:QCOLS], x8_d.ap()[:, :, 0:QCOLS])
        nc.sync.dma_start(H8[:, :, 0:1024], H8_d.ap()[:, :, 0:1024])
        for q in range(1, NQ):
            cs = slice(q * QCOLS, (q + 1) * QCOLS)
            nc.sync.dma_start(x8[:, :, cs], x8_d.ap()[:, :, cs])
        nc.sync.dma_start(H8[:, :, 1024:NHB * 512],
                          H8_d.ap()[:, :, 1024:NHB * 512])

        smnegs = []
        for c in range(CHUNKS):
            smneg = dbuf.tile([128, N_TOT], F16, name="smneg")
            smnegs.append(smneg)
            rsl = slice(c * 128, (c + 1) * 128)
            wset = set(win_blocks(c))
            for q in range(NQ):
                ps = pspool.tile([128, QCOLS], F32)
                for kk in range(KS // 2):
                    ks = slice(2 * kk, 2 * kk + 2)
                    for nb in range(QCOLS // 512):
                        B = q * 4 + nb
                        col = B * 512
                        nc.tensor.matmul(
                            ps[:, nb * 512:(nb + 1) * 512],
                            xn8[:, ks, rsl], x8[:, ks, col:col + 512],
                            start=(kk == 0),
                            stop=(kk == KS // 2 - 1 and B not in wset),
                            perf_mode=DR)
                # onehot matmuls only for the window blocks of this quarter
                for kk in range(KS // 2):
                    ks = slice(2 * kk, 2 * kk + 2)
                    for nb in range(QCOLS // 512):
                        B = q * 4 + nb
                        if B not in wset:
                            continue
                        hcol = H_MAP[B] * 512
                        nc.tensor.matmul(
                            ps[:, nb * 512:(nb + 1) * 512],
                            H8[:, ks, 512 + c * 128:512 + (c + 1) * 128],
                            H8[:, ks, hcol:hcol + 512],
                            start=False, stop=(kk == KS // 2 - 1),
                            perf_mode=DR)
                # evacuation PSUM f32 -> SBUF f16 (keeps 64x scale):
                # ACT does everything except the tail DEV cols of q3, which
                # DVE picks up in its idle window
                if q != NQ - 1:
                    nc.scalar.activation(smneg[:, q * QCOLS:(q + 1) * QCOLS],
                                         ps[:], Act.Copy, bias=0.0, scale=1.0)
                else:
                    nc.scalar.activation(
                        smneg[:, q * QCOLS:(q + 1) * QCOLS - DEV],
                        ps[:, 0:QCOLS - DEV], Act.Copy, bias=0.0, scale=1.0)
                    nc.vector.tensor_scalar(
                        smneg[:, (q + 1) * QCOLS - DEV:(q + 1) * QCOLS],
                        ps[:, QCOLS - DEV:QCOLS], 1.0, None, Alu.mult)
                if c == 0:
                    # ramp is DMA-bound and DVE idle: quarter-partial rowmin
                    # so chunk 0's chain starts the moment q3 lands
                    nc.vector.tensor_scalar(
                        junkD[:, q * QCOLS:(q + 1) * QCOLS],
                        smneg[:, q * QCOLS:(q + 1) * QCOLS], 0.0, 1e30,
                        Alu.add, Alu.min, accum_out=qmin[:, q:q + 1])

            # rowmin first: it feeds bneg -> relu-p, the longest chain
            if c == 0:
                nc.vector.tensor_reduce(stage[:, C_MIN:C_MIN + 1], qmin[:],
                                        mybir.AxisListType.X, Alu.min)
            else:
                nc.vector.tensor_scalar(junkD[:], smneg[:], 0.0, 1e30,
                                        Alu.add, Alu.min,
                                        accum_out=stage[:, C_MIN + c:C_MIN + c + 1])
            # rowmax over the exact class window only (positives >> negatives)
            for i, (lo, hi) in enumerate(win_pieces(c)):
                nc.vector.tensor_scalar(
                    junkD[:, lo:hi], smneg[:, lo:hi], 0.0, -1e30,
                    Alu.add, Alu.max,
                    accum_out=stage[:, C_MAX + 2 * c + i:C_MAX + 2 * c + i + 1])
            # bneg on Pool (feeds the ACT relu next chunk); a'/b' on DVE so
            # the count passes don't wait on a cross-engine hop
            nc.gpsimd.tensor_scalar(bneg[:, c:c + 1],
                                    stage[:, C_MIN + c:C_MIN + c + 1],
                                    -1.0, -SCL * (4.0 - MARGIN),
                                    Alu.mult, Alu.add)
            nc.vector.tensor_scalar(beta[:, c:c + 1],
                                    stage[:, C_MIN + c:C_MIN + c + 1],
                                    SCL * (4.0 - MARGIN), None, Alu.add)
            if len(win_pieces(c)) == 1:
                nc.vector.tensor_scalar(alpha[:, c:c + 1],
                                        stage[:, C_MAX + 2 * c:C_MAX + 2 * c + 1],
                                        -SCL * (4.0 - MARGIN), None, Alu.add)
            else:
                nc.vector.tensor_scalar(alpha[:, c:c + 1],
                                        stage[:, C_MAX + 2 * c:C_MAX + 2 * c + 1],
                                        stage[:, C_MAX + 2 * c + 1:C_MAX + 2 * c + 2],
                                        None, Alu.max)
                nc.vector.tensor_scalar(alpha[:, c:c + 1], alpha[:, c:c + 1],
                                        -SCL * (4.0 - MARGIN), None, Alu.add)
            # software pipeline: ACT relu-p (window) for chunk c-1 after
            # this chunk's evacs so it doesn't head-of-line block them
            if c >= 1:
                cp = c - 1
                for i, (lo, hi) in enumerate(win_pieces(cp)):
                    nc.scalar.activation(
                        junkA[:, lo:hi], smnegs[cp][:, lo:hi], Act.Relu,
                        bias=bneg[:, cp:cp + 1], scale=1.0,
                        accum_out=stage[:, C_PRELU + 2 * cp + i:
                                        C_PRELU + 2 * cp + i + 1])
            a_ap = alpha[:, c:c + 1]
            b_ap = beta[:, c:c + 1]
            last = c == CHUNKS - 1
            ncols = TSPL if last else N_TOT
            if last:
                nc.vector.tensor_scalar(aneg[:], a_ap, -1.0, None, Alu.mult)
            nc.vector.tensor_scalar(junkD[:, 0:ncols], smneg[:, 0:ncols],
                                    a_ap, 0.0, Alu.is_lt, Alu.add,
                                    accum_out=stage[:, C_NCNT + c:C_NCNT + c + 1])
            for i, (lo, hi) in enumerate(win_pieces(c)):
                nc.vector.tensor_scalar(
                    junkD[:, lo:hi], smneg[:, lo:hi], b_ap, 0.0,
                    Alu.is_gt, Alu.add,
                    accum_out=stage[:, C_PCNT + 2 * c + i:C_PCNT + 2 * c + i + 1])
            # nrelu' = W*a' - sum min(s',a')   (host does the W*a' part)
            nc.vector.tensor_scalar(junkD[:, 0:ncols], smneg[:, 0:ncols], a_ap,
                                    0.0, Alu.min, Alu.add,
                                    accum_out=stage[:, C_MINS + c:C_MINS + c + 1])
            if last:
                # tail split: ACT covers cols [TSPL:W] of the last chunk so
                # the final chunk's reductions run on both engines.
                # sum sign(s'-a') -> count part; sum relu(a'-s') -> nrelu part
                nc.scalar.activation(
                    junkA[:, TSPL:], smneg[:, TSPL:], Act.Sign,
                    bias=aneg[:], scale=1.0,
                    accum_out=stage[:, C_SGN:C_SGN + 1])
                nc.scalar.activation(
                    junkA[:, TSPL:], smneg[:, TSPL:], Act.Relu,
                    bias=a_ap, scale=-1.0,
                    accum_out=stage[:, C_NRELU:C_NRELU + 1])

        c = CHUNKS - 1
        for i, (lo, hi) in enumerate(win_pieces(c)):
            nc.scalar.activation(
                junkA[:, lo:hi], smnegs[c][:, lo:hi], Act.Relu,
                bias=bneg[:, c:c + 1], scale=1.0,
                accum_out=stage[:, C_PRELU + 2 * c + i:C_PRELU + 2 * c + i + 1])

        nc.sync.dma_start(st_d.ap(), stage[:])
    nc.compile()
    return nc


_NC_CACHE = None


def kernel(inputs, targets, _want_time=False, _trace=False):
    global _NC_CACHE
    x = np.asarray(inputs, dtype=np.float32)          # [N, D]
    tgt = np.asarray(targets).astype(np.int64)        # [N]

    # class-sorted permutation of rows/columns
    perm = np.argsort(tgt, kind="stable")
    xs = x[perm]
    ts_ = tgt[perm]
    assert np.bincount(tgt).max() <= MAX_CLASS, "class too large for window"

    xT = np.ascontiguousarray(xs.T)                   # [D, N] sorted cols
    x8g = np.ascontiguousarray(
        (8.0 * xT).reshape(KS, 128, N_TOT).transpose(1, 0, 2)
    ).astype(ml_dtypes.float8_e4m3)                   # [128, KS, N]
    H = np.zeros((D, N_TOT), dtype=np.float32)
    H[ts_, np.arange(N_TOT)] = 16.0
    H8g = np.ascontiguousarray(
        H.reshape(KS, 128, N_TOT).transpose(1, 0, 2)
    ).astype(ml_dtypes.float8_e4m3)

    if _NC_CACHE is None:
        _NC_CACHE = build_program()
    nc = _NC_CACHE

    in_maps = []
    for m in range(N_CORES):
        sh = m * ROWS
        x8m = np.roll(x8g, -sh, axis=2)
        xn8m = np.ascontiguousarray(
            (-x8m[:, :, 0:ROWS].astype(np.float32))).astype(
                ml_dtypes.float8_e4m3)
        # H8 window blocks in local coords -> global cols (sh + b*512) % N
        hw = np.concatenate(
            [np.take(H8g, (np.arange(b * 512, (b + 1) * 512) + sh) % N_TOT,
                     axis=2) for b in H_BLOCKS], axis=2)
        in_maps.append({"x8": np.ascontiguousarray(x8m),
                        "H8": np.ascontiguousarray(hw), "xn8": xn8m})

    res = run_bass_kernel_spmd(nc, in_maps, core_ids=list(range(N_CORES)),
                               trace=_trace)

    # ---- host finisher (rows are in sorted order; loss is order-invariant) --
    n = N_TOT
    W = float(N_TOT)
    minS = np.empty(n); maxS = np.empty(n)
    ncnt = np.empty(n); pcnt = np.empty(n)
    prelu = np.empty(n); minsum = np.empty(n)
    for m in range(N_CORES):
        st = np.asarray(res.results[m]["stage"], dtype=np.float64)
        for c in range(CHUNKS):
            rows = slice(m * ROWS + c * 128, m * ROWS + (c + 1) * 128)
            nb = len(win_pieces(c))
            minS[rows] = st[:, C_MIN + c] / SCL
            ncnt[rows] = st[:, C_NCNT + c]
            minsum[rows] = st[:, C_MINS + c] / SCL
            mx = st[:, C_MAX + 2 * c]
            if nb == 2:
                mx = np.maximum(mx, st[:, C_MAX + 2 * c + 1])
            maxS[rows] = mx / SCL
            pc = st[:, C_PCNT + 2 * c]
            pr = st[:, C_PRELU + 2 * c]
            if nb == 2:
                pc = pc + st[:, C_PCNT + 2 * c + 1]
                pr = pr + st[:, C_PRELU + 2 * c + 1]
            pcnt[rows] = pc
            prelu[rows] = pr / SCL

    ncnt = np.round(ncnt)
    pcnt = np.round(pcnt)
    alpha = maxS - (4.0 - MARGIN)
    beta = minS + (4.0 - MARGIN)
    nrelu = W * alpha - minsum
    # last chunk of each core: DVE covered cols [0:TSPL], ACT the rest
    for m in range(N_CORES):
        st = np.asarray(res.results[m]["stage"], dtype=np.float64)
        rows = slice(m * ROWS + (CHUNKS - 1) * 128, (m + 1) * ROWS)
        a_r = alpha[rows]
        ncnt[rows] = np.round(
            ncnt[rows] + ((N_TOT - TSPL) - st[:, C_SGN]) / 2.0)
        nrelu[rows] = (TSPL * a_r - minsum[rows]) + st[:, C_NRELU] / SCL
    # neg: kept s < alpha ; nrelu = alpha*ncnt - sum(s_kept); sim = -s
    neg_sum_sim = nrelu - alpha * ncnt
    # pos: kept s > beta ; prelu = sum(s_kept) - beta*pcnt; sim = 4 - s
    pos_sum_sim = 4.0 * pcnt - (prelu + beta * pcnt)

    pos_loss = (pcnt - pos_sum_sim) / np.maximum(pcnt, 1.0)
    neg_loss = neg_sum_sim / np.maximum(ncnt, 1.0)
    valid = ncnt >= 1.0
    loss = np.sum(np.where(valid, pos_loss + neg_loss, 0.0)) / n
    prec = np.sum(~valid) / n

    # last-row unmined stats: exact f64 closed form on host (original order)
    c_last = tgt[-1]
    xl = x[-1].astype(np.float64)
    x64 = x.astype(np.float64)
    same_l = tgt == c_last
    sum_all = x64.sum(axis=0) @ xl
    sum_same = x64[same_l].sum(axis=0) @ xl          # includes self
    self_sim = float(xl @ xl)
    include_self = np.float32(self_sim) < np.float32(1.0)
    pos_cnt_all = int(same_l.sum()) - 1 + (1 if include_self else 0)
    pos_sim_all = sum_same - (0.0 if include_self else self_sim)
    neg_cnt_all = int((~same_l).sum())
    neg_sim_all = sum_all - sum_same
    mean_pos_sim = pos_sim_all / max(pos_cnt_all, 1)
    mean_neg_sim = neg_sim_all / max(neg_cnt_all, 1)

    out = np.array([loss, prec, mean_pos_sim, mean_neg_sim], dtype=np.float32)
    if _want_time:
        return out, res
    return out
